# revision 13
# baseline (speedup 1.0000x reference)
"""Causal self-attention (B=2, T=2048, C=1024, H=16, D=64) on 8 trn2 cores.

Sharding: core = b*4 + hg  (data-parallel over batch b, tensor-parallel over
4 head-groups of 4 heads).  Each core computes q/k/v projections for its
256 head-dims, flash-style causal attention for its 4 heads, and a partial
output projection (its 256-column slice of Wp).  Partials are summed on the
host (the all-reduce), bias bp added there too.

Device layout notes (v2):
  - All matmul operands in bf16 (PSUM accumulation stays fp32).  bf16
    enables the compiler's fast-weight-load path (FWL) so LDWEIGHTS hides
    behind the previous matmul, and halves HBM/DVE traffic.
  - qT/kT stored as [d', t] with 2 heads packed per 128 partitions.
  - S tiles pack the HEAD PAIR: st = [128 k-chunk rows, 2*512] where the
    col halves are the two heads' S for the same (j, kc).  The two S
    matmuls have K=64 contraction at base partitions 0/64, so they land
    on disjoint PE row groups (auto tile_position) and run concurrently.
  - exp on ACT: one full-width [128, 1024] instruction per chunk
    (garbage cols left of the causal offset are computed but never read).
  - v stored [t, d'] with a ones-column per head (65 cols/head) so PV's
    output row 64 is the softmax denominator l[q] for free.
  - PV + normalize for a pair are deferred: pt chunks persist in SBUF,
    and the whole PV sweep is woven as PE filler into the NEXT pair's
    ACT-paced S/exp phase.  Diagonal triangle masks run on GpSimd
    (otherwise idle) to keep DVE load down.
  - Out-projection for block j is woven into block j+1 (not all deferred
    to the end), flattening the power draw that previously tripped the
    HAM clock throttle for the whole final block.
"""
import numpy as np
from contextlib import ExitStack

B, T, C, H, D = 2, 2048, 1024, 16, 64
HLOC = 4            # heads per core
CLOC = HLOC * D     # 256 head-dims per core
VW = HLOC * 65      # v width with ones-columns: 260
N_CORES = 8
TQ = 512            # q tile width
KC = 128            # k chunk
NCC = C // 128      # 8 contraction chunks
NJ = T // TQ        # 4 q blocks

_CACHE = {}


def build_nc(with_qk_bias=True):
    import concourse.tile as tile
    from concourse import bacc, mybir

    f32 = mybir.dt.float32
    bf16 = mybir.dt.bfloat16
    Exp = mybir.ActivationFunctionType.Exp

    nc = bacc.Bacc("TRN2", target_bir_lowering=False, debug=False,
                   num_devices=N_CORES)
    xT = nc.dram_tensor("xT", [C, T], bf16, kind="ExternalInput").ap()
    wqT = nc.dram_tensor("wqT", [C, CLOC], bf16, kind="ExternalInput").ap()
    wkT = nc.dram_tensor("wkT", [C, CLOC], bf16, kind="ExternalInput").ap()
    wvT = nc.dram_tensor("wvT", [C, VW], bf16, kind="ExternalInput").ap()
    wpT = nc.dram_tensor("wpT", [CLOC, C], bf16, kind="ExternalInput").ap()
    bq = nc.dram_tensor("bq", [1, CLOC], bf16, kind="ExternalInput").ap()
    bk = nc.dram_tensor("bk", [1, CLOC], bf16, kind="ExternalInput").ap()
    bv = nc.dram_tensor("bv", [1, VW], bf16, kind="ExternalInput").ap()
    ones = nc.dram_tensor("ones", [1, TQ], bf16, kind="ExternalInput").ap()
    tri = nc.dram_tensor("tri", [KC, KC], bf16, kind="ExternalInput").ap()
    po = nc.dram_tensor("po", [T, C], bf16, kind="ExternalOutput").ap()

    with tile.TileContext(nc) as tc, ExitStack() as ctx:
        persist = ctx.enter_context(tc.tile_pool(name="persist", bufs=1))
        pt_pool = ctx.enter_context(tc.tile_pool(name="pt", bufs=34))
        stage = ctx.enter_context(tc.tile_pool(name="stage", bufs=4))
        norm_pool = ctx.enter_context(tc.tile_pool(name="norm", bufs=3))
        ps_small = ctx.enter_context(
            tc.tile_pool(name="ps_small", bufs=2, space="PSUM"))
        ps_st = ctx.enter_context(
            tc.tile_pool(name="ps_st", bufs=2, space="PSUM"))
        ps_yt = ctx.enter_context(
            tc.tile_pool(name="ps_yt", bufs=2, space="PSUM"))

        # ---- persistent SBUF tensors + loads ----
        xT_sb = [persist.tile([128, T], bf16, tag=f"xT{c}", name=f"xT{c}") for c in range(NCC)]
        wq_sb = [persist.tile([128, CLOC], bf16, tag=f"wq{c}", name=f"wq{c}") for c in range(NCC)]
        wk_sb = [persist.tile([128, CLOC], bf16, tag=f"wk{c}", name=f"wk{c}") for c in range(NCC)]
        wv_sb = [persist.tile([128, VW], bf16, tag=f"wv{c}", name=f"wv{c}") for c in range(NCC)]
        wp_sb = [persist.tile([128, C], bf16, tag=f"wp{m}", name=f"wp{m}") for m in range(2)]
        bq_sb = persist.tile([1, CLOC], bf16, tag="bq")
        bk_sb = persist.tile([1, CLOC], bf16, tag="bk")
        bv_sb = persist.tile([1, VW], bf16, tag="bv")
        ones_sb = persist.tile([1, TQ], bf16, tag="ones")
        tri_sb = persist.tile([KC, KC], bf16, tag="tri")
        qT_sb = [persist.tile([128, T], bf16, tag=f"qT{m}", name=f"qT{m}") for m in range(2)]
        kT_sb = [persist.tile([128, T], bf16, tag=f"kT{m}", name=f"kT{m}") for m in range(2)]
        v_sb = [persist.tile([128, VW], bf16, tag=f"v{t}", name=f"v{t}") for t in range(T // 128)]
        yT_sb = [persist.tile([128, T], bf16, tag=f"yT{m}", name=f"yT{m}") for m in range(2)]

        # Preamble DMA issue is split across the Sync and Scalar HWDGE
        # sequencers (Scalar is idle until the first exp) — the per-issue
        # cost (~0.5us each), not HBM bandwidth, paced the old preamble.
        # The 16 first-chain transfers (x block 0 + Wq) issue first, 8 on
        # each sequencer.
        for c in range(NCC):
            sl = slice(c * 128, (c + 1) * 128)
            nc.sync.dma_start(xT_sb[c][:, 0:TQ], xT[sl, 0:TQ])
            nc.scalar.dma_start(wq_sb[c][:], wqT[sl, :])
        for c in range(NCC):
            sl = slice(c * 128, (c + 1) * 128)
            nc.sync.dma_start(wk_sb[c][:], wkT[sl, :])
            nc.scalar.dma_start(wv_sb[c][:], wvT[sl, :])
        for c in range(NCC):
            sl = slice(c * 128, (c + 1) * 128)
            nc.sync.dma_start(xT_sb[c][:, TQ:T], xT[sl, TQ:T])
        for m in range(2):
            nc.scalar.dma_start(wp_sb[m][:], wpT[m * 128:(m + 1) * 128, :])
        nc.scalar.dma_start(tri_sb[:], tri[:])
        nc.scalar.dma_start(ones_sb[:], ones[:])
        nc.scalar.dma_start(bv_sb[:], bv[:])
        if with_qk_bias:
            nc.scalar.dma_start(bq_sb[:], bq[:])
            nc.scalar.dma_start(bk_sb[:], bk[:])

        # ---- projection / out-projection pieces (PE filler units) ----
        def proj_qk(w_sb, b_sb, dst, m, t):
            tsl = slice(t * TQ, (t + 1) * TQ)
            msl = slice(m * 128, (m + 1) * 128)
            ps = ps_small.tile([128, TQ], f32, tag="ps_small")
            for c in range(NCC):
                nc.tensor.matmul(ps[:], w_sb[c][:, msl], xT_sb[c][:, tsl],
                                 start=(c == 0),
                                 stop=(c == NCC - 1 and not with_qk_bias))
            if with_qk_bias:
                nc.tensor.matmul(ps[:], b_sb[0:1, msl], ones_sb[0:1, :],
                                 start=False, stop=True)
            nc.vector.tensor_copy(dst[m][:, tsl], ps[:])

        def proj_v(tt):
            ttsl = slice(tt * 128, tt * 128 + 128)
            ps = ps_small.tile([128, VW], f32, tag="ps_small")
            for c in range(NCC):
                nc.tensor.matmul(ps[:], xT_sb[c][:, ttsl], wv_sb[c][:],
                                 start=(c == 0), stop=False)
            # always emitted: supplies the ones-columns (+ v bias)
            nc.tensor.matmul(ps[:], ones_sb[0:1, 0:128], bv_sb[:],
                             start=False, stop=True)
            nc.vector.tensor_copy(v_sb[tt][:], ps[:])

        def proj_pieces(t, qk=True, v=True):
            out = []
            if qk:
                for w_sb, b_sb, dst in ((wq_sb, bq_sb, qT_sb), (wk_sb, bk_sb, kT_sb)):
                    for m in range(2):
                        out.append(lambda w=w_sb, b=b_sb, d=dst, mm=m:
                                   proj_qk(w, b, d, mm, t))
            if v:
                for tt in range(t * 4, t * 4 + 4):
                    out.append(lambda x=tt: proj_v(x))
            return out

        def outproj_piece(tt, do, on_scalar=False):
            ttsl = slice(tt * 128, (tt + 1) * 128)
            dsl = slice(do * TQ, (do + 1) * TQ)
            ops = ps_small.tile([128, TQ], f32, tag="ps_small")
            for m2 in range(2):
                nc.tensor.matmul(ops[:], yT_sb[m2][:, ttsl],
                                 wp_sb[m2][:, dsl],
                                 start=(m2 == 0), stop=(m2 == 1))
            so = stage.tile([128, TQ], bf16, tag="so")
            # epilogue copies ride the then-idle Scalar engine (GpSimd has
            # no PSUM port; mid-kernel ACT is busy with exp)
            if on_scalar:
                nc.scalar.copy(so[:], ops[:])
            else:
                nc.vector.tensor_copy(so[:], ops[:])
            nc.sync.dma_start(po[ttsl, dsl], so[:])

        def outproj_pieces(j, on_scalar=False):
            return [lambda t=tt, d=do: outproj_piece(t, d, on_scalar)
                    for tt in range(4 * j, 4 * j + 4) for do in range(2)]

        # ---- attention primitives ----
        def s_exp_chunk(j, hp, kc):
            """S matmuls for both heads of pair hp, chunk kc + full exp.

            Returns (pt, coff).  The two S matmuls contract over K=64 at
            base partitions 0/64 -> disjoint row groups -> concurrent."""
            coff = max(0, kc * KC - j * TQ)
            st = ps_st.tile([128, 2 * TQ], f32, tag="st")
            for hh in range(2):
                pr = hh * 64
                nc.tensor.matmul(
                    st[:, hh * TQ + coff:(hh + 1) * TQ],
                    kT_sb[hp][pr:pr + 64, kc * KC:(kc + 1) * KC],
                    qT_sb[hp][pr:pr + 64, j * TQ + coff:(j + 1) * TQ],
                    start=True, stop=True)
            pt = pt_pool.tile([128, 2 * TQ], bf16, tag="pt",
                              name=f"pt{j}_{hp}_{kc}")
            nc.scalar.activation(pt[:], st[:], Exp, scale=0.125)
            if kc >= 4 * j:   # diagonal chunk: mask the 128-wide triangle
                for hh in range(2):
                    lo = hh * TQ + coff
                    nc.gpsimd.tensor_mul(pt[:, lo:lo + KC], pt[:, lo:lo + KC],
                                         tri_sb[:])
            return pt, coff

        def pv_piece(j, h, kc, nk, pts, yt):
            pt, coff = pts[kc]
            hh = h % 2
            nc.tensor.matmul(
                yt[0:65, coff:TQ] if coff else yt[:],
                v_sb[kc][:, h * 65:(h + 1) * 65],
                pt[:, hh * TQ + coff:(hh + 1) * TQ],
                start=(kc == 0), stop=(kc == nk - 1))

        def normalize(j, h, yt):
            """yT[h slice, j] = yt[0:64] * broadcast(1/l).

            The partition broadcast is a stride-0-AP SBUF->SBUF DMA, not a
            PE ones-matmul — keeps the PE out of the normalize chain."""
            m, pr = h // 2, (h % 2) * 64
            l_sb = norm_pool.tile([1, TQ], f32, tag="l")
            nc.vector.tensor_copy(l_sb[:], yt[64:65, :])
            bc_sb = stage.tile([64, TQ], f32, tag="bc")
            nc.gpsimd.partition_broadcast(bc_sb[:], l_sb[0:1, :])
            bi_sb = stage.tile([64, TQ], f32, tag="bi")
            nc.vector.reciprocal_approx_fast(bi_sb[:], bc_sb[:])
            nc.vector.tensor_mul(yT_sb[m][pr:pr + 64, j * TQ:(j + 1) * TQ],
                                 yt[0:64, :], bi_sb[:])

        def pair_drain_pieces(j, hp, nk, pts):
            """PV sweeps + normalizes for a finished pair, as filler units."""
            out = []
            yts = {}
            for hh in range(2):
                h = 2 * hp + hh

                def first(h=h):
                    yts[h] = ps_yt.tile([65, TQ], f32, tag="yt",
                                        name=f"yt{j}_{h}")
                    pv_piece(j, h, 0, nk, pts, yts[h])
                out.append(first)
                for kc in range(1, nk):
                    out.append(lambda h=h, kc=kc:
                               pv_piece(j, h, kc, nk, pts, yts[h]))
                out.append(lambda h=h: normalize(j, h, yts.pop(h)))
            return out

        def rr_merge(a, b):
            """Proportional round-robin merge preserving each list's order."""
            out, ia, ib = [], 0, 0
            na, nb = len(a), len(b)
            while ia < na or ib < nb:
                if ib >= nb or (ia < na and ia * (nb + 1) <= ib * (na + 1)):
                    out.append(a[ia])
                    ia += 1
                else:
                    out.append(b[ib])
                    ib += 1
            return out

        # ---- main schedule ----
        for piece in proj_pieces(0, v=False):   # prologue: q/k of block 0
            piece()

        prev = None     # (j, hp, nk, pts) of the pair awaiting PV drain
        for j in range(NJ):
            nk = 4 * (j + 1)
            for hp in range(2):
                drain = pair_drain_pieces(*prev) if prev is not None else []
                other = []
                if j == 0 and hp == 0:
                    other += proj_pieces(0, qk=False)   # deferred v(0)
                if j + 1 < NJ:
                    pp = proj_pieces(j + 1)
                    half = len(pp) // 2
                    other += pp[:half] if hp == 0 else pp[half:]
                if hp == 1 and j >= 1:
                    other += outproj_pieces(j - 1)
                extras = rr_merge(drain, other)
                pts = {}
                ei = 0
                for kc in range(nk):
                    pts[kc] = s_exp_chunk(j, hp, kc)
                    want = (kc + 1) * len(extras) // nk
                    while ei < want:
                        extras[ei]()
                        ei += 1
                while ei < len(extras):
                    extras[ei]()
                    ei += 1
                prev = (j, hp, nk, pts)

        for piece in pair_drain_pieces(*prev):      # epilogue
            piece()
        for piece in outproj_pieces(NJ - 1, on_scalar=True):
            piece()
    nc.compile()
    return nc


def make_in_maps(x, Wq, bq, Wk, bk, Wv, bv, Wp, bp):
    import ml_dtypes
    bf16 = ml_dtypes.bfloat16

    x = np.asarray(x, np.float32)
    Wq, Wk, Wv, Wp = (np.asarray(w, np.float32) for w in (Wq, Wk, Wv, Wp))
    bq, bk, bv = (np.asarray(b, np.float32) for b in (bq, bk, bv))

    ones = np.ones((1, TQ), bf16)
    kp = np.arange(KC)[:, None]
    qf = np.arange(KC)[None, :]
    tri = (qf >= kp).astype(bf16)

    in_maps = []
    for core in range(N_CORES):
        b = core // 4
        hg = core % 4
        rows = slice(hg * CLOC, (hg + 1) * CLOC)
        wv_aug = np.zeros((C, VW), np.float32)
        bv_aug = np.zeros((1, VW), np.float32)
        for h in range(HLOC):
            wsl = slice(hg * CLOC + h * D, hg * CLOC + (h + 1) * D)
            wv_aug[:, h * 65:h * 65 + D] = Wv[wsl, :].T
            bv_aug[0, h * 65 + D] = 1.0
            bv_aug[0, h * 65:h * 65 + D] = bv[wsl]
        in_maps.append({
            "xT": np.ascontiguousarray(x[b].T).astype(bf16),
            "wqT": np.ascontiguousarray(Wq[rows, :].T).astype(bf16),
            "wkT": np.ascontiguousarray(Wk[rows, :].T).astype(bf16),
            "wvT": wv_aug.astype(bf16),
            "wpT": np.ascontiguousarray(Wp[:, rows].T).astype(bf16),
            "bq": np.ascontiguousarray(bq[rows][None, :]).astype(bf16),
            "bk": np.ascontiguousarray(bk[rows][None, :]).astype(bf16),
            "bv": bv_aug.astype(bf16),
            "ones": ones,
            "tri": tri,
        })
    return in_maps


def kernel(x, Wq, bq, Wk, bk, Wv, bv, Wp, bp):
    from concourse.bass_utils import run_bass_kernel_spmd

    with_qk_bias = bool(np.any(np.asarray(bq)) or np.any(np.asarray(bk)))
    key = ("nc", with_qk_bias)
    if key not in _CACHE:
        _CACHE[key] = build_nc(with_qk_bias)
    nc = _CACHE[key]
    in_maps = make_in_maps(x, Wq, bq, Wk, bk, Wv, bv, Wp, bp)
    res = run_bass_kernel_spmd(nc, in_maps, core_ids=list(range(N_CORES)))
    out = np.zeros((B, T, C), np.float32)
    for core in range(N_CORES):
        out[core // 4] += res.results[core]["po"].astype(np.float32)
    out += np.asarray(bp, np.float32)[None, None, :]
    return out


# revision 14
# speedup vs baseline: 1.5396x; 1.5396x over previous
"""Causal self-attention (B=2, T=2048, C=1024, H=16, D=64) on 8 trn2 cores.

Sharding: core = b*4 + hg  (data-parallel over batch b, tensor-parallel over
4 head-groups of 4 heads).  Each core computes q/k/v projections for its
256 head-dims, flash-style causal attention for its 4 heads, and a partial
output projection (its 256-column slice of Wp).  Partials are summed on the
host (the all-reduce), bias bp added there too.

Device layout notes (v2):
  - All matmul operands in bf16 (PSUM accumulation stays fp32).  bf16
    enables the compiler's fast-weight-load path (FWL) so LDWEIGHTS hides
    behind the previous matmul, and halves HBM/DVE traffic.
  - qT/kT stored as [d', t] with 2 heads packed per 128 partitions.
  - S tiles pack the HEAD PAIR: st = [128 k-chunk rows, 2*512] where the
    col halves are the two heads' S for the same (j, kc).  The two S
    matmuls have K=64 contraction at base partitions 0/64, so they land
    on disjoint PE row groups (auto tile_position) and run concurrently.
  - exp on ACT: one full-width [128, 1024] instruction per chunk
    (garbage cols left of the causal offset are computed but never read).
  - v stored [t, d'] with a ones-column per head (65 cols/head) so PV's
    output row 64 is the softmax denominator l[q] for free.
  - PV + normalize for a pair are deferred: pt chunks persist in SBUF,
    and the whole PV sweep is woven as PE filler into the NEXT pair's
    ACT-paced S/exp phase.  Diagonal triangle masks run on GpSimd
    (otherwise idle) to keep DVE load down.
  - Out-projection for block j is woven into block j+1 (not all deferred
    to the end), flattening the power draw that previously tripped the
    HAM clock throttle for the whole final block.
"""
import numpy as np
from contextlib import ExitStack

B, T, C, H, D = 2, 2048, 1024, 16, 64
HLOC = 4            # heads per core
CLOC = HLOC * D     # 256 head-dims per core
VW = HLOC * 65      # v width with ones-columns: 260
N_CORES = 8
TQ = 512            # q tile width
KC = 128            # k chunk
NCC = C // 128      # 8 contraction chunks
NJ = T // TQ        # 4 q blocks

_CACHE = {}


def build_nc(with_qk_bias=True):
    import concourse.tile as tile
    from concourse import bacc, mybir

    f32 = mybir.dt.float32
    bf16 = mybir.dt.bfloat16
    Exp = mybir.ActivationFunctionType.Exp

    nc = bacc.Bacc("TRN2", target_bir_lowering=False, debug=False,
                   num_devices=N_CORES)
    xT = nc.dram_tensor("xT", [C, T], bf16, kind="ExternalInput").ap()
    wqT = nc.dram_tensor("wqT", [C, CLOC], bf16, kind="ExternalInput").ap()
    wkT = nc.dram_tensor("wkT", [C, CLOC], bf16, kind="ExternalInput").ap()
    wvT = nc.dram_tensor("wvT", [C, VW], bf16, kind="ExternalInput").ap()
    wpT = nc.dram_tensor("wpT", [CLOC, C], bf16, kind="ExternalInput").ap()
    bq = nc.dram_tensor("bq", [1, CLOC], bf16, kind="ExternalInput").ap()
    bk = nc.dram_tensor("bk", [1, CLOC], bf16, kind="ExternalInput").ap()
    bv = nc.dram_tensor("bv", [1, VW], bf16, kind="ExternalInput").ap()
    ones = nc.dram_tensor("ones", [1, TQ], bf16, kind="ExternalInput").ap()
    tri = nc.dram_tensor("tri", [KC, KC], bf16, kind="ExternalInput").ap()
    po = nc.dram_tensor("po", [T, C], bf16, kind="ExternalOutput").ap()

    with tile.TileContext(nc) as tc, ExitStack() as ctx:
        persist = ctx.enter_context(tc.tile_pool(name="persist", bufs=1))
        pt_pool = ctx.enter_context(tc.tile_pool(name="pt", bufs=34))
        stage = ctx.enter_context(tc.tile_pool(name="stage", bufs=4))
        norm_pool = ctx.enter_context(tc.tile_pool(name="norm", bufs=3))
        ps_small = ctx.enter_context(
            tc.tile_pool(name="ps_small", bufs=2, space="PSUM"))
        ps_st = ctx.enter_context(
            tc.tile_pool(name="ps_st", bufs=2, space="PSUM"))
        ps_yt = ctx.enter_context(
            tc.tile_pool(name="ps_yt", bufs=2, space="PSUM"))

        # ---- persistent SBUF tensors + loads ----
        xT_sb = [persist.tile([128, T], bf16, tag=f"xT{c}", name=f"xT{c}") for c in range(NCC)]
        wq_sb = [persist.tile([128, CLOC], bf16, tag=f"wq{c}", name=f"wq{c}") for c in range(NCC)]
        wk_sb = [persist.tile([128, CLOC], bf16, tag=f"wk{c}", name=f"wk{c}") for c in range(NCC)]
        wv_sb = [persist.tile([128, VW], bf16, tag=f"wv{c}", name=f"wv{c}") for c in range(NCC)]
        wp_sb = [persist.tile([128, C], bf16, tag=f"wp{m}", name=f"wp{m}") for m in range(2)]
        bq_sb = persist.tile([1, CLOC], bf16, tag="bq")
        bk_sb = persist.tile([1, CLOC], bf16, tag="bk")
        bv_sb = persist.tile([1, VW], bf16, tag="bv")
        ones_sb = persist.tile([1, TQ], bf16, tag="ones")
        tri_sb = persist.tile([KC, KC], bf16, tag="tri")
        qT_sb = [persist.tile([128, T], bf16, tag=f"qT{m}", name=f"qT{m}") for m in range(2)]
        kT_sb = [persist.tile([128, T], bf16, tag=f"kT{m}", name=f"kT{m}") for m in range(2)]
        v_sb = [persist.tile([128, VW], bf16, tag=f"v{t}", name=f"v{t}") for t in range(T // 128)]
        yT_sb = [persist.tile([128, T], bf16, tag=f"yT{m}", name=f"yT{m}") for m in range(2)]

        # Preamble DMA issue is split across the Sync and Scalar HWDGE
        # sequencers (Scalar is idle until the first exp) — the per-issue
        # cost (~0.5us each), not HBM bandwidth, paced the old preamble.
        # The 16 first-chain transfers (x block 0 + Wq) issue first, 8 on
        # each sequencer.
        for c in range(NCC):
            sl = slice(c * 128, (c + 1) * 128)
            nc.sync.dma_start(xT_sb[c][:, 0:TQ], xT[sl, 0:TQ])
            nc.scalar.dma_start(wq_sb[c][:], wqT[sl, :])
        for c in range(NCC):
            sl = slice(c * 128, (c + 1) * 128)
            nc.sync.dma_start(wk_sb[c][:], wkT[sl, :])
            nc.scalar.dma_start(wv_sb[c][:], wvT[sl, :])
        for c in range(NCC):
            sl = slice(c * 128, (c + 1) * 128)
            nc.sync.dma_start(xT_sb[c][:, TQ:T], xT[sl, TQ:T])
        for m in range(2):
            nc.scalar.dma_start(wp_sb[m][:], wpT[m * 128:(m + 1) * 128, :])
        nc.scalar.dma_start(tri_sb[:], tri[:])
        nc.scalar.dma_start(ones_sb[:], ones[:])
        nc.scalar.dma_start(bv_sb[:], bv[:])
        if with_qk_bias:
            nc.scalar.dma_start(bq_sb[:], bq[:])
            nc.scalar.dma_start(bk_sb[:], bk[:])

        # ---- projection / out-projection pieces (PE filler units) ----
        def proj_qk(w_sb, b_sb, dst, m, t):
            tsl = slice(t * TQ, (t + 1) * TQ)
            msl = slice(m * 128, (m + 1) * 128)
            ps = ps_small.tile([128, TQ], f32, tag="ps_small")
            for c in range(NCC):
                nc.tensor.matmul(ps[:], w_sb[c][:, msl], xT_sb[c][:, tsl],
                                 start=(c == 0),
                                 stop=(c == NCC - 1 and not with_qk_bias))
            if with_qk_bias:
                nc.tensor.matmul(ps[:], b_sb[0:1, msl], ones_sb[0:1, :],
                                 start=False, stop=True)
            nc.vector.tensor_copy(dst[m][:, tsl], ps[:])

        def proj_v(tt):
            ttsl = slice(tt * 128, tt * 128 + 128)
            ps = ps_small.tile([128, VW], f32, tag="ps_small")
            for c in range(NCC):
                nc.tensor.matmul(ps[:], xT_sb[c][:, ttsl], wv_sb[c][:],
                                 start=(c == 0), stop=False)
            # always emitted: supplies the ones-columns (+ v bias)
            nc.tensor.matmul(ps[:], ones_sb[0:1, 0:128], bv_sb[:],
                             start=False, stop=True)
            nc.vector.tensor_copy(v_sb[tt][:], ps[:])

        def proj_pieces(t, qk=True, v=True):
            out = []
            if qk:
                for w_sb, b_sb, dst in ((wq_sb, bq_sb, qT_sb), (wk_sb, bk_sb, kT_sb)):
                    for m in range(2):
                        out.append(lambda w=w_sb, b=b_sb, d=dst, mm=m:
                                   proj_qk(w, b, d, mm, t))
            if v:
                for tt in range(t * 4, t * 4 + 4):
                    out.append(lambda x=tt: proj_v(x))
            return out

        def outproj_piece(tt, do, on_scalar=False):
            ttsl = slice(tt * 128, (tt + 1) * 128)
            dsl = slice(do * TQ, (do + 1) * TQ)
            ops = ps_small.tile([128, TQ], f32, tag="ps_small")
            for m2 in range(2):
                nc.tensor.matmul(ops[:], yT_sb[m2][:, ttsl],
                                 wp_sb[m2][:, dsl],
                                 start=(m2 == 0), stop=(m2 == 1))
            so = stage.tile([128, TQ], bf16, tag="so")
            # epilogue copies ride the then-idle Scalar engine (GpSimd has
            # no PSUM port; mid-kernel ACT is busy with exp)
            if on_scalar:
                nc.scalar.copy(so[:], ops[:])
            else:
                nc.vector.tensor_copy(so[:], ops[:])
            nc.sync.dma_start(po[ttsl, dsl], so[:])

        def outproj_pieces(j, on_scalar=False):
            return [lambda t=tt, d=do: outproj_piece(t, d, on_scalar)
                    for tt in range(4 * j, 4 * j + 4) for do in range(2)]

        # ---- attention primitives ----
        def s_exp_chunk(j, hp, kc):
            """S matmuls for both heads of pair hp, chunk kc + full exp.

            Returns (pt, coff).  The two S matmuls contract over K=64 at
            base partitions 0/64 -> disjoint row groups -> concurrent."""
            coff = max(0, kc * KC - j * TQ)
            st = ps_st.tile([128, 2 * TQ], f32, tag="st")
            for hh in range(2):
                pr = hh * 64
                nc.tensor.matmul(
                    st[:, hh * TQ + coff:(hh + 1) * TQ],
                    kT_sb[hp][pr:pr + 64, kc * KC:(kc + 1) * KC],
                    qT_sb[hp][pr:pr + 64, j * TQ + coff:(j + 1) * TQ],
                    start=True, stop=True)
            pt = pt_pool.tile([128, 2 * TQ], bf16, tag="pt",
                              name=f"pt{j}_{hp}_{kc}")
            nc.scalar.activation(pt[:], st[:], Exp, scale=0.125)
            if kc >= 4 * j:   # diagonal chunk: mask the 128-wide triangle
                for hh in range(2):
                    lo = hh * TQ + coff
                    nc.gpsimd.tensor_mul(pt[:, lo:lo + KC], pt[:, lo:lo + KC],
                                         tri_sb[:])
            return pt, coff

        def pv_piece(j, h, kc, nk, pts, yt):
            pt, coff = pts[kc]
            hh = h % 2
            nc.tensor.matmul(
                yt[0:65, coff:TQ] if coff else yt[:],
                v_sb[kc][:, h * 65:(h + 1) * 65],
                pt[:, hh * TQ + coff:(hh + 1) * TQ],
                start=(kc == 0), stop=(kc == nk - 1))

        def normalize(j, h, yt):
            """yT[h slice, j] = yt[0:64] * broadcast(1/l)."""
            m, pr = h // 2, (h % 2) * 64
            l_sb = norm_pool.tile([1, TQ], bf16, tag="l")
            nc.vector.tensor_copy(l_sb[:], yt[64:65, :])
            bc_ps = ps_small.tile([64, TQ], f32, tag="ps_small")
            nc.tensor.matmul(bc_ps[:], ones_sb[0:1, 0:64], l_sb[:],
                             start=True, stop=True)
            bc_sb = stage.tile([64, TQ], f32, tag="bc")
            nc.vector.reciprocal_approx_fast(bc_sb[:], bc_ps[:])
            nc.vector.tensor_mul(yT_sb[m][pr:pr + 64, j * TQ:(j + 1) * TQ],
                                 yt[0:64, :], bc_sb[:])

        def pair_drain_pieces(j, hp, nk, pts):
            """PV sweeps + normalizes for a finished pair, as filler units."""
            out = []
            yts = {}
            for hh in range(2):
                h = 2 * hp + hh

                def first(h=h):
                    yts[h] = ps_yt.tile([65, TQ], f32, tag="yt",
                                        name=f"yt{j}_{h}")
                    pv_piece(j, h, 0, nk, pts, yts[h])
                out.append(first)
                for kc in range(1, nk):
                    out.append(lambda h=h, kc=kc:
                               pv_piece(j, h, kc, nk, pts, yts[h]))
                out.append(lambda h=h: normalize(j, h, yts.pop(h)))
            return out

        def rr_merge(a, b):
            """Proportional round-robin merge preserving each list's order."""
            out, ia, ib = [], 0, 0
            na, nb = len(a), len(b)
            while ia < na or ib < nb:
                if ib >= nb or (ia < na and ia * (nb + 1) <= ib * (na + 1)):
                    out.append(a[ia])
                    ia += 1
                else:
                    out.append(b[ib])
                    ib += 1
            return out

        # ---- main schedule ----
        for piece in proj_pieces(0, v=False):   # prologue: q/k of block 0
            piece()

        prev = None     # (j, hp, nk, pts) of the pair awaiting PV drain
        for j in range(NJ):
            nk = 4 * (j + 1)
            for hp in range(2):
                drain = pair_drain_pieces(*prev) if prev is not None else []
                other = []
                if j == 0 and hp == 0:
                    other += proj_pieces(0, qk=False)   # deferred v(0)
                if j + 1 < NJ:
                    pp = proj_pieces(j + 1)
                    half = len(pp) // 2
                    other += pp[:half] if hp == 0 else pp[half:]
                if hp == 1 and j >= 1:
                    other += outproj_pieces(j - 1)
                extras = rr_merge(drain, other)
                pts = {}
                ei = 0
                for kc in range(nk):
                    pts[kc] = s_exp_chunk(j, hp, kc)
                    want = (kc + 1) * len(extras) // nk
                    while ei < want:
                        extras[ei]()
                        ei += 1
                while ei < len(extras):
                    extras[ei]()
                    ei += 1
                prev = (j, hp, nk, pts)

        for piece in pair_drain_pieces(*prev):      # epilogue
            piece()
        for piece in outproj_pieces(NJ - 1, on_scalar=True):
            piece()
    nc.compile()
    return nc


def make_in_maps(x, Wq, bq, Wk, bk, Wv, bv, Wp, bp):
    import ml_dtypes
    bf16 = ml_dtypes.bfloat16

    x = np.asarray(x, np.float32)
    Wq, Wk, Wv, Wp = (np.asarray(w, np.float32) for w in (Wq, Wk, Wv, Wp))
    bq, bk, bv = (np.asarray(b, np.float32) for b in (bq, bk, bv))

    ones = np.ones((1, TQ), bf16)
    kp = np.arange(KC)[:, None]
    qf = np.arange(KC)[None, :]
    tri = (qf >= kp).astype(bf16)

    in_maps = []
    for core in range(N_CORES):
        b = core // 4
        hg = core % 4
        rows = slice(hg * CLOC, (hg + 1) * CLOC)
        wv_aug = np.zeros((C, VW), np.float32)
        bv_aug = np.zeros((1, VW), np.float32)
        for h in range(HLOC):
            wsl = slice(hg * CLOC + h * D, hg * CLOC + (h + 1) * D)
            wv_aug[:, h * 65:h * 65 + D] = Wv[wsl, :].T
            bv_aug[0, h * 65 + D] = 1.0
            bv_aug[0, h * 65:h * 65 + D] = bv[wsl]
        in_maps.append({
            "xT": np.ascontiguousarray(x[b].T).astype(bf16),
            "wqT": np.ascontiguousarray(Wq[rows, :].T).astype(bf16),
            "wkT": np.ascontiguousarray(Wk[rows, :].T).astype(bf16),
            "wvT": wv_aug.astype(bf16),
            "wpT": np.ascontiguousarray(Wp[:, rows].T).astype(bf16),
            "bq": np.ascontiguousarray(bq[rows][None, :]).astype(bf16),
            "bk": np.ascontiguousarray(bk[rows][None, :]).astype(bf16),
            "bv": bv_aug.astype(bf16),
            "ones": ones,
            "tri": tri,
        })
    return in_maps


def kernel(x, Wq, bq, Wk, bk, Wv, bv, Wp, bp):
    from concourse.bass_utils import run_bass_kernel_spmd

    with_qk_bias = bool(np.any(np.asarray(bq)) or np.any(np.asarray(bk)))
    key = ("nc", with_qk_bias)
    if key not in _CACHE:
        _CACHE[key] = build_nc(with_qk_bias)
    nc = _CACHE[key]
    in_maps = make_in_maps(x, Wq, bq, Wk, bk, Wv, bv, Wp, bp)
    res = run_bass_kernel_spmd(nc, in_maps, core_ids=list(range(N_CORES)))
    out = np.zeros((B, T, C), np.float32)
    for core in range(N_CORES):
        out[core // 4] += res.results[core]["po"].astype(np.float32)
    out += np.asarray(bp, np.float32)[None, None, :]
    return out
